# revision 13
# baseline (speedup 1.0000x reference)
"""Trainium2 Bass kernel for nn_MessageAggregator_74440373174623.

GNN metapath message aggregation with per-destination-node segment softmax:
  a = lrelu((features @ attn1_w.T)[node_idx] + metapath_embedding @ attn2.T)
  attn = segment_softmax(a, node_idx); h = segment_sum(attn * emb)
  out = elu(h)  -> [N, H*D]

Sharding: edges sorted by destination node; node ranges split edge-balanced
across 8 cores. Within a core, nodes are packed into blocks of exactly 128
node-slots (empty filler slots allowed) holding <= 2176 edge slots
(17 groups x 128). Segment softmax is shift-invariant, so no segment max is
needed (fp32 exp range suffices for randn-scale scores).

Per block (all on device):
  - one-hot P[e,w] = (idxrel[e] == w) built by DVE (iota + is_equal, bf16)
  - Pt = P^T via PE transpose (+psum->sbuf copy on DVE/ACT)
  - t0 = a1+a2 accumulated in PSUM by three matmuls per 128-edge group:
      Pt^T @ s_hi + Pt^T @ s_lo  (s split into two bf16 halves for accuracy)
      + embT_g^T @ attn2T
  - a = lrelu(t0) (ACT), ex = exp(a) (ACT, bf16)
  - rhs = [ex | ex*embc] (DVE broadcast multiply)
  - psum[128,260] += P_g^T @ rhs_g  (denominators | weighted sums)
  - h = elu(hraw / denom) per node slot -> out rows [128b, 128b+128)

Host work is layout only: sort/shard/pad/transpose + index tables.
"""

import numpy as np
import ml_dtypes
from contextlib import ExitStack

D = 64
H = 4
ALPHA = 0.2
NCORES = 8
GROUP = 128
GPB = 17
EPB = GROUP * GPB  # 2176
RHSW = H + H * D   # 260

bf16 = ml_dtypes.bfloat16


# ---------------------------------------------------------------- host prep
def _prep(metapath_embedding, features, attn1_w, attn2, node_idx):
    E = node_idx.shape[0]
    N = features.shape[0]
    idx = np.asarray(node_idx).astype(np.int64)
    counts = np.bincount(idx, minlength=N)
    cum = np.cumsum(counts)

    bounds = [0]
    for k in range(1, NCORES):
        bounds.append(int(np.searchsorted(cum, k * E / NCORES)))
    bounds.append(N)

    order = np.argsort(idx, kind="stable")
    sidx = idx[order]
    estart = [int(np.searchsorted(sidx, bounds[k])) for k in range(NCORES)] + [E]

    cores = []
    NBs = []
    for k in range(NCORES):
        n0, n1 = bounds[k], bounds[k + 1]
        NL = n1 - n0
        ecnt = counts[n0:n1]
        blocks = []  # (first_real_node_local, n_real, n_edges)
        p = 0
        while p < NL:
            w = 0
            ne = 0
            while p + w < NL and w < 128 and ne + ecnt[p + w] <= EPB:
                ne += int(ecnt[p + w])
                w += 1
            assert w > 0, "node degree exceeds block capacity"
            blocks.append((p, w, ne))
            p += w
        cores.append(dict(n0=n0, n1=n1, NL=NL, blocks=blocks,
                          e0=estart[k], e1=estart[k + 1]))
        NBs.append(len(blocks))

    NB = max(NBs)
    NPp = NB * 128
    EP = NB * EPB
    G = NB * GPB

    in_maps = []
    for k, c in enumerate(cores):
        n0 = c["n0"]
        eptr = c["e0"]
        slotmap = np.full(EP, -1, dtype=np.int64)   # edge slot -> sorted pos
        slotrel = np.full(EP, -1, dtype=np.int64)   # edge slot -> node slot rel
        featT = np.zeros((D, NPp), dtype=np.float32)
        binfo = []
        for b, (r0, w, ne) in enumerate(c["blocks"]):
            s0 = b * EPB
            slotmap[s0:s0 + ne] = np.arange(eptr, eptr + ne)
            nn = sidx[eptr:eptr + ne] - n0          # local real node ids
            slotrel[s0:s0 + ne] = nn - r0
            featT[:, 128 * b:128 * b + w] = \
                features[n0 + r0:n0 + r0 + w].T
            eptr += ne
            binfo.append((r0, w))
        assert eptr == c["e1"]

        valid = slotmap >= 0
        gemb = np.zeros((EP, D), dtype=np.float32)
        gemb[valid] = metapath_embedding[order[slotmap[valid]]]

        s_r = np.arange(EP) % GROUP
        s_g = np.arange(EP) // GROUP

        embT = np.ascontiguousarray(gemb.T.astype(bf16))
        embcT = np.zeros((GROUP, G * D), dtype=bf16)
        embcT.reshape(GROUP, G, D)[s_r, s_g] = gemb.astype(bf16)

        idxrel = np.full((GROUP, G), -1.0, dtype=bf16)
        idxrel[s_r, s_g] = slotrel.astype(np.float32).astype(bf16)

        in_maps.append(dict(
            embT=embT, embcT=embcT, idxrel=idxrel, featT=featT,
            attn1T=np.ascontiguousarray(attn1_w.T.astype(np.float32)),
            attn2T=np.ascontiguousarray(attn2.T.astype(bf16)),
            _binfo=binfo, _n0=n0, _n1=c["n1"],
        ))

    meta = dict(NB=NB, NPp=NPp, EP=EP, G=G, N=N)
    return in_maps, meta


# ------------------------------------------------------------- kernel build
def _build(NB, num_devices=NCORES, debug_taps=False, repeats=1):
    import concourse.bacc as bacc
    import concourse.mybir as mybir
    import concourse.tile as tile

    dt = mybir.dt
    G = NB * GPB
    EP = NB * EPB
    NPp = NB * 128

    nc = bacc.Bacc(
        "TRN2", target_bir_lowering=False, debug=False, num_devices=num_devices
    )

    embT_d = nc.dram_tensor("embT", [D, EP], dt.bfloat16, kind="ExternalInput")
    embcT_d = nc.dram_tensor("embcT", [GROUP, G * D], dt.bfloat16,
                             kind="ExternalInput")
    idxrel_d = nc.dram_tensor("idxrel", [GROUP, G], dt.bfloat16,
                              kind="ExternalInput")
    featT_d = nc.dram_tensor("featT", [D, NPp], dt.float32, kind="ExternalInput")
    attn1T_d = nc.dram_tensor("attn1T", [D, H], dt.float32, kind="ExternalInput")
    attn2T_d = nc.dram_tensor("attn2T", [D, H], dt.bfloat16, kind="ExternalInput")
    out_d = nc.dram_tensor("out", [NPp, H * D], dt.float32,
                           kind="ExternalOutput")
    if debug_taps:
        dbg_t0 = nc.dram_tensor("dbg_t0", [GROUP, GPB * H], dt.float32,
                                kind="ExternalOutput")
        dbg_rhs = nc.dram_tensor("dbg_rhs", [GROUP, GPB * RHSW], dt.bfloat16,
                                 kind="ExternalOutput")
        dbg_P = nc.dram_tensor("dbg_P", [GROUP, EPB], dt.bfloat16,
                               kind="ExternalOutput")
        dbg_s = nc.dram_tensor("dbg_s", [GROUP, NB * 2 * H], dt.bfloat16,
                               kind="ExternalOutput")

    f32 = dt.float32
    b16 = dt.bfloat16
    AF = mybir.ActivationFunctionType
    MAX = mybir.AluOpType.max
    MULT = mybir.AluOpType.mult
    ISEQ = mybir.AluOpType.is_equal

    with tile.TileContext(nc) as tc, ExitStack() as ctx:
        const = ctx.enter_context(tc.tile_pool(name="const", bufs=1))
        featp = ctx.enter_context(tc.tile_pool(name="featp", bufs=3))
        sps = ctx.enter_context(tc.tile_pool(name="sps", bufs=2, space="PSUM"))
        embp = ctx.enter_context(tc.tile_pool(name="embp", bufs=3))
        embcp = ctx.enter_context(tc.tile_pool(name="embcp", bufs=3))
        tps = ctx.enter_context(tc.tile_pool(name="tps", bufs=2, space="PSUM"))
        t0ps_p = ctx.enter_context(tc.tile_pool(name="t0ps", bufs=2,
                                                space="PSUM"))
        work = ctx.enter_context(tc.tile_pool(name="work", bufs=3))
        rhsp = ctx.enter_context(tc.tile_pool(name="rhsp", bufs=3))
        pp = ctx.enter_context(tc.tile_pool(name="pp", bufs=2))
        ptp = ctx.enter_context(tc.tile_pool(name="ptp", bufs=2))
        hps = ctx.enter_context(tc.tile_pool(name="hps", bufs=2, space="PSUM"))
        outp = ctx.enter_context(tc.tile_pool(name="outp", bufs=3))

        # ---- constants ----
        attn1T = const.tile([D, H], f32)
        nc.sync.dma_start(attn1T[:], attn1T_d[:])
        attn2T = const.tile([D, H], b16)
        nc.sync.dma_start(attn2T[:], attn2T_d[:])

        iota_i = const.tile([GROUP, GROUP], dt.int32, tag="iota_i")
        nc.gpsimd.iota(iota_i[:], pattern=[[1, GROUP]], base=0,
                       channel_multiplier=0)
        iotaC = const.tile([GROUP, GROUP], b16, tag="iotaC")
        nc.vector.tensor_copy(iotaC[:], iota_i[:])

        idn_i = const.tile([GROUP, GROUP], dt.int32, tag="idn_i")
        nc.gpsimd.iota(idn_i[:], pattern=[[1, GROUP]], base=0,
                       channel_multiplier=-1)
        I128 = const.tile([GROUP, GROUP], b16, tag="I128")
        nc.vector.tensor_scalar(I128[:], idn_i[:], 0, None, op0=ISEQ)

        idxrel_sb = const.tile([GROUP, G], b16)
        nc.sync.dma_start(idxrel_sb[:], idxrel_d[:])

        # s table: per block 8 bf16 cols = [hi(4) | lo(4)]
        s_sb = const.tile([GROUP, NB * 2 * H], b16, tag="s_sb")

        # ---- s-pass ----
        for b in range(NB * repeats):
            b = b % NB
            ft = featp.tile([D, GROUP], f32)
            nc.sync.dma_start(ft[:], featT_d[:, b * 128:(b + 1) * 128])
            ps = sps.tile([GROUP, H], f32)
            nc.tensor.matmul(ps[:], ft[:], attn1T[:], start=True, stop=True)
            sc = featp.tile([GROUP, H], f32, tag="sc")
            nc.vector.tensor_copy(sc[:], ps[:])
            shi = s_sb[:, 8 * b:8 * b + 4]
            nc.vector.tensor_copy(shi, sc[:])
            shi32 = featp.tile([GROUP, H], f32, tag="shi32")
            nc.vector.tensor_copy(shi32[:], shi)
            dlo = featp.tile([GROUP, H], f32, tag="dlo")
            nc.vector.tensor_sub(dlo[:], sc[:], shi32[:])
            nc.vector.tensor_copy(s_sb[:, 8 * b + 4:8 * b + 8], dlo[:])
        if debug_taps:
            nc.sync.dma_start(dbg_s[:], s_sb[:])

        # ---- main blocks ----
        for bb in range(NB * repeats):
            b = bb % NB
            e0 = b * EPB
            embT_t = embp.tile([D, EPB], b16)
            nc.sync.dma_start(embT_t[:], embT_d[:, e0:e0 + EPB])
            embcT_t = embcp.tile([GROUP, GPB * D], b16)
            nc.sync.dma_start(embcT_t[:],
                              embcT_d[:, b * GPB * D:(b + 1) * GPB * D])

            # one-hot P for the whole block
            P = pp.tile([GROUP, EPB], b16)
            p3 = P[:].rearrange("p (g w) -> p g w", w=GROUP)
            pin0 = iotaC[:].unsqueeze(1).broadcast_to([GROUP, GPB, GROUP])
            pin1 = (idxrel_sb[:, b * GPB:(b + 1) * GPB].unsqueeze(2)
                    .broadcast_to([GROUP, GPB, GROUP]))
            nc.vector.tensor_tensor(p3, pin0, pin1, op=ISEQ)

            # Pt = P^T via PE transpose, batched psum->sbuf copies
            Pt = ptp.tile([GROUP, EPB], b16)
            for q, g0 in enumerate(range(0, GPB, 4)):
                gn = min(4, GPB - g0)
                pst = tps.tile([GROUP, 4 * GROUP], b16, tag="pst")
                for j in range(gn):
                    g = g0 + j
                    nc.tensor.transpose(
                        pst[:, j * GROUP:(j + 1) * GROUP],
                        P[:, g * GROUP:(g + 1) * GROUP],
                        I128[:],
                    )
                dst = Pt[:, g0 * GROUP:(g0 + gn) * GROUP]
                src = pst[:, :gn * GROUP]
                if q % 2 == 0:
                    nc.vector.tensor_copy(dst, src)
                else:
                    nc.scalar.copy(dst, src)

            # t0 = a1(hi)+a1(lo)+a2 accumulated per group into PSUM
            t0ps = t0ps_p.tile([GROUP, GPB * H], f32)
            shi = s_sb[:, 8 * b:8 * b + 4]
            slo = s_sb[:, 8 * b + 4:8 * b + 8]
            for g in range(GPB):
                o = t0ps[:, g * H:(g + 1) * H]
                lt = Pt[:, g * GROUP:(g + 1) * GROUP]
                nc.tensor.matmul(o, lt, shi, start=True, stop=False)
                nc.tensor.matmul(o, lt, slo, start=False, stop=False)
                nc.tensor.matmul(o, embT_t[:, g * GROUP:(g + 1) * GROUP],
                                 attn2T[:], start=False, stop=True)

            # a = lrelu(t0); ex = exp(a) into rhs ex slots
            t1 = work.tile([GROUP, GPB * H], f32, tag="t1")
            nc.vector.tensor_scalar_mul(t1[:], t0ps[:], ALPHA)
            am = work.tile([GROUP, GPB * H], f32, tag="am")
            nc.vector.tensor_tensor(am[:], t0ps[:], t1[:], op=MAX)

            rhs = rhsp.tile([GROUP, GPB * RHSW], b16)
            rhs3 = rhs[:].rearrange("p (g c) -> p g c", c=RHSW)
            nc.scalar.activation(
                rhs3[:, :, 0:H],
                am[:].rearrange("p (g h) -> p g h", h=H),
                AF.Exp,
            )
            wout = rhs3[:, :, H:RHSW].rearrange("p g (h d) -> p g h d", d=D)
            win0 = (embcT_t[:].rearrange("p (g d) -> p g d", d=D)
                    .unsqueeze(2).broadcast_to([GROUP, GPB, H, D]))
            win1 = (rhs3[:, :, 0:H].unsqueeze(3)
                    .broadcast_to([GROUP, GPB, H, D]))
            nc.vector.tensor_tensor(wout, win0, win1, op=MULT)

            if debug_taps and b == 0:
                nc.sync.dma_start(dbg_P[:], P[:])
                nc.sync.dma_start(dbg_rhs[:], rhs[:])
                am2 = work.tile([GROUP, GPB * H], f32, tag="am2")
                nc.vector.tensor_copy(am2[:], t0ps[:])
                nc.sync.dma_start(dbg_t0[:], am2[:])

            # scatter: psum[128, 260] accumulates [denom | h]
            psH = hps.tile([GROUP, RHSW], f32)
            for g in range(GPB):
                nc.tensor.matmul(
                    psH[:],
                    P[:, g * GROUP:(g + 1) * GROUP],
                    rhs[:, g * RHSW:(g + 1) * RHSW],
                    start=(g == 0),
                    stop=(g == GPB - 1),
                )

            dn = work.tile([GROUP, H], f32, tag="dn")
            nc.vector.tensor_scalar_add(dn[:], psH[:, 0:H], 1e-30)
            rd = work.tile([GROUP, H], f32, tag="rd")
            nc.vector.reciprocal(rd[:], dn[:])

            hsc = outp.tile([GROUP, H * D], f32, tag="hsc")
            nc.vector.tensor_tensor(
                hsc[:].rearrange("p (h d) -> p h d", d=D),
                psH[:, H:RHSW].rearrange("p (h d) -> p h d", d=D),
                rd[:].unsqueeze(2).broadcast_to([GROUP, H, D]),
                op=MULT,
            )
            tm = outp.tile([GROUP, H * D], f32, tag="tm")
            nc.vector.tensor_scalar_min(tm[:], hsc[:], 0.0)
            ex1 = outp.tile([GROUP, H * D], f32, tag="ex1")
            nc.scalar.activation(ex1[:], tm[:], AF.Exp)
            em1 = outp.tile([GROUP, H * D], f32, tag="em1")
            nc.vector.tensor_scalar_sub(em1[:], ex1[:], 1.0)
            ho = outp.tile([GROUP, H * D], f32, tag="ho")
            nc.vector.tensor_tensor(ho[:], em1[:], hsc[:], op=MAX)

            nc.sync.dma_start(out_d[b * 128:(b + 1) * 128, :], ho[:])

    nc.compile()
    return nc


_LAST_RESULTS = {}


def kernel(**inputs) -> np.ndarray:
    from concourse.bass_utils import run_bass_kernel_spmd

    inputs = {k: np.asarray(v) for k, v in inputs.items()}
    in_maps, meta = _prep(**inputs)
    nc = _build(meta["NB"])

    dev_maps = [
        {k: v for k, v in m.items() if not k.startswith("_")} for m in in_maps
    ]
    res = run_bass_kernel_spmd(nc, dev_maps, list(range(NCORES)))
    _LAST_RESULTS["res"] = res
    _LAST_RESULTS["meta"] = meta

    N = meta["N"]
    full = np.zeros((N, H * D), dtype=np.float32)
    for k, m in enumerate(in_maps):
        od = np.asarray(res.results[k]["out"])
        n0 = m["_n0"]
        for b, (r0, w) in enumerate(m["_binfo"]):
            full[n0 + r0:n0 + r0 + w] = od[b * 128:b * 128 + w]
    return full
